# revision 38
# baseline (speedup 1.0000x reference)
"""Deformable-Conv (DCNv1) + SyncBN + LeakyReLU Trainium2 kernel.

Self-contained: shards the full inputs over 8 NeuronCores (data-parallel over
(batch, row-half); BN stats all-reduced on-device), runs one SPMD Bass/Tile
kernel via run_bass_kernel_spmd, and reassembles the full output.

Per-core pipeline:
  P0 load full image (cast fp16) into padded SBUF; build row-major transposed
     copy x_pad_T[ppos, c] in DRAM (the gather source). Separately load the
     core's 66-row halo window for the offset conv.
  P1 offset conv (9 accumulating fp16 matmuls per 4-row group) + PE
     transposes -> offsets in [w-partition, (row, tap)] layout.
  P2 elementwise chain (DVE fp32): bilinear indices + border-folded weights.
  P3 wrapped-index build for dma_gather (double PE-transpose trick).
  P4 per (8-row group, tap): dma_gather 2x2 patches as row-pairs, 4-term
     weighted combine (ACT scale-mul + 3 DVE scalar_tensor_tensor), PE
     transpose to [c, pos], fp16 matmuls accumulating over taps in PSUM;
     ACT copy-with-accum for BN stats.
  P5 BN stats AllReduce (2x256 floats) + scale/bias solve.
  P6 fused affine+LeakyReLU (Relu homogeneity split) + store fp32.
"""
import sys

sys.path.insert(0, "/opt/trn_rl_repo")

import numpy as np

import concourse.bacc as bacc
import concourse.mybir as mybir
from concourse import tile
from concourse.ap import AP
from concourse.tile_rust import add_dep_helper

ALU = mybir.AluOpType
DT = mybir.dt
AF = mybir.ActivationFunctionType

N_CORES = 8
B, C, O, H, W = 4, 128, 256, 128, 128
KS, NT = 3, 9
ROWS = 64                 # output rows per core
NG2, R8 = 8, 8            # main loop: 8 groups of 8 rows
Hp = H + 2                # 130
PADF = 134 * 128          # 17152: padded flat image size (>= 130*130 + margin)
NPOS = ROWS * W           # 8192
EPS = 1e-5
LEAK = 0.1
MAGIC = float(3 << 22)    # 1.5 * 2^23: fp32 round-to-int magic
NCALLS = NG2 * NT * 2     # 144 dma_gather calls
CH = ROWS * NT            # 576: elementwise-chain free size
CW = NCALLS * R8          # 1152: C-matrix free size
NCK = CW // 128           # 9 chunks

DX = np.repeat(np.arange(-1, 2), 3).astype(np.float32)
DY = np.tile(np.arange(-1, 2), 3).astype(np.float32)


def build_kernel(with_collective=True):
    nc = bacc.Bacc("TRN2", target_bir_lowering=False)

    # ---- I/O ----
    x_img = nc.dram_tensor("x_img", [C, H * W], DT.float32, kind="ExternalInput")
    x_win = nc.dram_tensor("x_win", [C, 66 * W], DT.float32, kind="ExternalInput")
    pwT_d = nc.dram_tensor("pwT", [NT, C, 2 * NT], DT.float16, kind="ExternalInput")
    pb_d = nc.dram_tensor("pb", [2 * NT, 1], DT.float32, kind="ExternalInput")
    wT_d = nc.dram_tensor("wT", [NT, C, O], DT.float16, kind="ExternalInput")
    ax_d = nc.dram_tensor("Ax", [128, CH], DT.float32, kind="ExternalInput")
    by_d = nc.dram_tensor("By", [128, CH], DT.float32, kind="ExternalInput")
    gam_d = nc.dram_tensor("gamma2", [128, 2], DT.float32, kind="ExternalInput")
    bet_d = nc.dram_tensor("beta2", [128, 2], DT.float32, kind="ExternalInput")
    idf_d = nc.dram_tensor("identf", [128, 128], DT.float32, kind="ExternalInput")
    idh_d = nc.dram_tensor("identh", [128, 128], DT.float16, kind="ExternalInput")

    out_d = nc.dram_tensor("out", [2, 128, NPOS], DT.float32, kind="ExternalOutput")

    xpt = nc.dram_tensor("xpt", [PADF * C], DT.float16)   # flat row-major [row, c]
    cc_in = nc.dram_tensor("cc_in", [128, 4], DT.float32)
    cc_out = nc.dram_tensor("cc_out", [128, 4], DT.float32)

    taps = [(ky, kx) for ky in range(3) for kx in range(3)]

    with tile.TileContext(nc) as tc:
        with tc.tile_pool(name="pp", bufs=1) as pp, \
             tc.tile_pool(name="pbig", bufs=1) as pbig, \
             tc.tile_pool(name="pch", bufs=15) as pch, \
             tc.tile_pool(name="pw4", bufs=1) as pw4, \
             tc.tile_pool(name="pg", bufs=3) as pg, \
             tc.tile_pool(name="pst", bufs=2) as pst, \
             tc.tile_pool(name="pps", bufs=2, space="PSUM") as pps, \
             tc.tile_pool(name="ppacc", bufs=2, space="PSUM") as ppacc:

            # ---------------- constants ----------------
            pw_sb = pp.tile([C, NT * 2 * NT], DT.float16, tag="pw")
            nc.sync.dma_start(pw_sb[:].rearrange("c (t m) -> c t m", m=2 * NT),
                              pwT_d[:].transpose([1, 0, 2]))
            pb_sb = pp.tile([2 * NT, 1], DT.float32, tag="pb")
            nc.sync.dma_start(pb_sb[:], pb_d[:])
            wt_sb = pp.tile([C, NT * O], DT.float16, tag="wt")
            nc.sync.dma_start(wt_sb[:].rearrange("c (t o) -> c t o", o=O),
                              wT_d[:].transpose([1, 0, 2]))
            ax_sb = pp.tile([128, CH], DT.float32, tag="ax")
            nc.sync.dma_start(ax_sb[:], ax_d[:])
            by_sb = pp.tile([128, CH], DT.float32, tag="by")
            nc.sync.dma_start(by_sb[:], by_d[:])
            gam_sb = pp.tile([128, 2], DT.float32, tag="gam")
            nc.sync.dma_start(gam_sb[:], gam_d[:])
            bet_sb = pp.tile([128, 2], DT.float32, tag="bet")
            nc.sync.dma_start(bet_sb[:], bet_d[:])
            idf = pp.tile([128, 128], DT.float32, tag="idf")
            nc.sync.dma_start(idf[:], idf_d[:])
            idh = pp.tile([128, 128], DT.float16, tag="idh")
            nc.sync.dma_start(idh[:], idh_d[:])

            # halo window for the offset conv first (unblocks P1 early)
            xwin = pbig.tile([C, 66 * 130], DT.float16, tag="xwin")
            nc.vector.memset(xwin[:], 0.0)
            for hb in range(2):
                win_int = AP(xwin.tensor, xwin[:].offset + 1 + hb * 33 * 130,
                             [xwin[:].ap[0], [130, 33], [1, W]])
                nc.gpsimd.dma_start(
                    out=win_int,
                    in_=x_win[:, hb * 33 * W:(hb + 1) * 33 * W]
                        .rearrange("c (h w) -> c h w", w=W))

            # ---------------- P0: padded fp16 image -> xpt (DRAM) ------------
            xph = pbig.tile([C, PADF], DT.float16, tag="xpad")
            nc.vector.memset(xph[:], 0.0)
            for hb in range(2):
                interior = AP(xph.tensor, xph[:].offset + 131 + hb * 64 * 130,
                              [xph[:].ap[0], [130, H // 2], [1, W]])
                nc.gpsimd.dma_start(
                    out=interior,
                    in_=x_img[:, hb * 64 * W:(hb + 1) * 64 * W]
                        .rearrange("c (h w) -> c h w", w=W))

            nchunk = PADF // 128            # 134
            xpt_stores = []
            stg = pbig.tile([128, nchunk * 128 // 2], DT.float16, tag="stg")
            pieces = [(i * nchunk // 8, (i + 1) * nchunk // 8) for i in range(8)]
            for p0, p1 in pieces:
                for ck in range(p0, p1):
                    j = ck % (nchunk // 2)
                    px0 = pps.tile([128, 128], DT.float16, tag="tph", name="px0")
                    nc.tensor.transpose(out=px0[:],
                                        in_=xph[:, ck * 128:(ck + 1) * 128],
                                        identity=idh[:])
                    if ck % 2 == 0:
                        nc.scalar.copy(stg[:, j * 128:(j + 1) * 128], px0[:])
                    else:
                        nc.vector.tensor_copy(stg[:, j * 128:(j + 1) * 128],
                                              px0[:])
                j0 = p0 % (nchunk // 2)
                dst = AP(xpt, p0 * 128 * C,
                         [[C, 128], [128 * C, p1 - p0], [1, C]])
                st = nc.sync.dma_start(
                    out=dst,
                    in_=stg[:, j0 * 128:(j0 + (p1 - p0)) * 128]
                        .rearrange("p (j c) -> p j c", c=C))
                xpt_stores.append(st)

            # ---------------- P1: offset conv -> offT[w, (row, m)] -----------
            offT = pw4.tile([128, ROWS * 2 * NT], DT.float32, tag="offT")
            pwr = pw_sb[:].rearrange("c (t m) -> c t m", m=2 * NT)
            for g in range(ROWS // 4):
                ps_off = pps.tile([2 * NT, 512], DT.float32, tag="tp")
                for t, (ky, kx) in enumerate(taps):
                    base = (g * 4 + ky) * 130 + kx
                    rhs = AP(xwin.tensor, xwin[:].offset + base,
                             [xwin[:].ap[0], [130, 4], [1, W]])
                    nc.tensor.matmul(ps_off[:], lhsT=pwr[:, t], rhs=rhs,
                                     start=(t == 0), stop=(t == 8))
                offc = pch.tile([2 * NT, 512], DT.float32, tag="ch")
                nc.scalar.activation(out=offc[:], in_=ps_off[:], func=AF.Identity,
                                     bias=pb_sb[:], scale=1.0)
                ps_t = pps.tile([128, 4 * 2 * NT], DT.float32, tag="tp")
                for r in range(4):
                    nc.tensor.transpose(out=ps_t[:, r * 2 * NT:(r + 1) * 2 * NT],
                                        in_=offc[:, r * 128:(r + 1) * 128],
                                        identity=idf[:2 * NT, :2 * NT])
                nc.vector.tensor_copy(offT[:, g * 4 * 2 * NT:(g + 1) * 4 * 2 * NT],
                                      ps_t[:])

            # ---------------- P2+P3 per row-half: chain + wrapped idx ------
            CHH = CH // 2          # 288 chain cols per half
            CWH = CW // 2          # 576 C cols per half
            offv = offT[:].rearrange("p (r m) -> p r m", m=2 * NT)
            wlt = pw4.tile([128, CH], DT.float32, tag="wlt")
            wlb = pw4.tile([128, CH], DT.float32, tag="wlb")
            wrt = pw4.tile([128, CH], DT.float32, tag="wrt")
            wrb = pw4.tile([128, CH], DT.float32, tag="wrb")
            cmat = pw4.tile([128, CW], DT.float32, tag="cmat")
            tsb = pw4.tile([128, 10 * 128], DT.float32, tag="tsb")
            wrap = pw4.tile([128, NCALLS * 64], DT.int16, tag="wrap")

            for half in range(2):
                r0 = half * (ROWS // 2)
                cs = slice(half * CHH, (half + 1) * CHH)

                def cht():
                    return pch.tile([128, CHH], DT.float32, tag="ch", name="cht")

                ox = cht()
                nc.vector.tensor_copy(
                    ox[:].rearrange("p (r n) -> p r n", n=NT),
                    offv[:, r0:r0 + ROWS // 2, 0:NT])
                oy = cht()
                nc.vector.tensor_copy(
                    oy[:].rearrange("p (r n) -> p r n", n=NT),
                    offv[:, r0:r0 + ROWS // 2, NT:2 * NT])
                px = cht()
                nc.vector.tensor_tensor(out=px[:], in0=ox[:], in1=ax_sb[:, cs],
                                        op=ALU.add)
                py = cht()
                nc.vector.tensor_tensor(out=py[:], in0=oy[:], in1=by_sb[:, cs],
                                        op=ALU.add)

                def floor_(v):
                    fl = cht()
                    nc.vector.tensor_scalar(out=fl[:], in0=v[:], scalar1=MAGIC,
                                            scalar2=MAGIC, op0=ALU.add,
                                            op1=ALU.subtract)
                    g_ = cht()
                    nc.vector.tensor_tensor(out=g_[:], in0=fl[:], in1=v[:],
                                            op=ALU.is_gt)
                    nc.vector.tensor_tensor(out=fl[:], in0=fl[:], in1=g_[:],
                                            op=ALU.subtract)
                    return fl

                fx = floor_(px)
                fy = floor_(py)

                def clip_lo_hi(v):
                    q0 = cht()
                    nc.vector.tensor_scalar(out=q0[:], in0=v[:], scalar1=0.0,
                                            scalar2=129.0, op0=ALU.max,
                                            op1=ALU.min)
                    q1 = cht()
                    nc.vector.tensor_scalar(out=q1[:], in0=v[:], scalar1=-1.0,
                                            scalar2=1.0, op0=ALU.max,
                                            op1=ALU.add)
                    nc.vector.tensor_scalar(out=q1[:], in0=q1[:], scalar1=129.0,
                                            scalar2=None, op0=ALU.min)
                    return q0, q1

                qltx, qrbx = clip_lo_hi(fx)
                qlty, qrby = clip_lo_hi(fy)
                pcx = cht()
                nc.vector.tensor_scalar(out=pcx[:], in0=px[:], scalar1=0.0,
                                        scalar2=129.0, op0=ALU.max, op1=ALU.min)
                pcy = cht()
                nc.vector.tensor_scalar(out=pcy[:], in0=py[:], scalar1=0.0,
                                        scalar2=129.0, op0=ALU.max, op1=ALU.min)

                def weights(qlt, qrb, pc):
                    a0 = cht()
                    nc.vector.scalar_tensor_tensor(out=a0[:], in0=qlt[:],
                                                   scalar=1.0, in1=pc[:],
                                                   op0=ALU.add,
                                                   op1=ALU.subtract)
                    a1 = cht()
                    nc.vector.scalar_tensor_tensor(out=a1[:], in0=pc[:],
                                                   scalar=1.0, in1=qrb[:],
                                                   op0=ALU.add,
                                                   op1=ALU.subtract)
                    eq = cht()
                    nc.vector.tensor_tensor(out=eq[:], in0=qrb[:], in1=qlt[:],
                                            op=ALU.is_equal)
                    t = cht()
                    nc.vector.tensor_tensor(out=t[:], in0=eq[:], in1=a1[:],
                                            op=ALU.mult)
                    nc.vector.tensor_tensor(out=a0[:], in0=a0[:], in1=t[:],
                                            op=ALU.add)
                    nc.vector.tensor_scalar(out=eq[:], in0=eq[:], scalar1=-1.0,
                                            scalar2=1.0, op0=ALU.mult,
                                            op1=ALU.add)
                    nc.vector.tensor_tensor(out=a1[:], in0=a1[:], in1=eq[:],
                                            op=ALU.mult)
                    return a0, a1

                a0, a1 = weights(qltx, qrbx, pcx)
                b0, b1 = weights(qlty, qrby, pcy)

                nc.vector.tensor_tensor(out=wlt[:, cs], in0=a0[:], in1=b0[:],
                                        op=ALU.mult)
                nc.vector.tensor_tensor(out=wlb[:, cs], in0=a0[:], in1=b1[:],
                                        op=ALU.mult)
                nc.vector.tensor_tensor(out=wrt[:, cs], in0=a1[:], in1=b0[:],
                                        op=ALU.mult)
                nc.vector.tensor_tensor(out=wrb[:, cs], in0=a1[:], in1=b1[:],
                                        op=ALU.mult)

                idx0 = cht()
                nc.vector.scalar_tensor_tensor(out=idx0[:], in0=qltx[:],
                                               scalar=130.0, in1=qlty[:],
                                               op0=ALU.mult, op1=ALU.add)
                idx1 = cht()
                nc.vector.tensor_scalar(out=idx1[:], in0=idx0[:], scalar1=130.0,
                                        scalar2=None, op0=ALU.add)

                # C matrix for this half: col = call2*16 + r*8 + jj (global)
                ccs = slice(half * CWH, (half + 1) * CWH)
                cview = cmat[:, ccs].rearrange("p (g n r j) -> p g n r j",
                                               g=NG2 // 2, n=NT, r=2)
                for rsel, isrc in ((0, idx0), (1, idx1)):
                    sview = isrc[:].rearrange("p (g j n) -> p g j n",
                                              g=NG2 // 2, j=R8)
                    nc.vector.tensor_copy(cview[:, :, :, rsel, :],
                                          sview.transpose([0, 1, 3, 2]))

                # chunks of this half's C columns
                base = half * CWH
                bounds = list(range(0, CWH, 128)) + [CWH]
                for nb, (lo, hi) in enumerate(zip(bounds[:-1], bounds[1:])):
                    cksz = hi - lo
                    cbase = base + lo
                    ci = half * 5 + nb
                    ps = pps.tile([128, 128], DT.float32, tag="tp", name="psT2")
                    nc.tensor.transpose(out=ps[:cksz, :],
                                        in_=cmat[:, cbase:cbase + cksz],
                                        identity=idf[:])
                    nc.scalar.copy(tsb[:cksz, ci * 128:(ci + 1) * 128],
                                   ps[:cksz, :])
                    for a in range(8):
                        wa = pps.tile([16, 128], DT.float32, tag="tp", name="wa")
                        nc.tensor.transpose(
                            out=wa[:, :cksz],
                            in_=tsb[:cksz, ci * 128 + 16 * a:ci * 128 + 16 * a + 16],
                            identity=idf[:cksz, :cksz])
                        dstv = AP(wrap.tensor, wrap[:].offset
                                  + (cbase // 16) * 128 + a,
                                  [[wrap[:].ap[0][0], 16],
                                   [128, cksz // 16], [8, 16]])
                        nc.vector.tensor_copy(
                            dstv,
                            wa[:, :cksz].rearrange("p (c j) -> p c j", j=16))
                for cgrp in range(1, 8):
                    wsl = slice(half * 36 * 128, (half + 1) * 36 * 128)
                    nc.sync.dma_start(out=wrap[16 * cgrp:16 * (cgrp + 1), wsl],
                                      in_=wrap[0:16, wsl])

            # ---------------- P4: gather + combine + matmul ------------------
            src_ap = AP(xpt, 0, [[C, PADF - 2], [1, 2 * C]])
            out_sb = pbig.tile([128, 2 * NPOS], DT.float16, tag="xpad")
            sums = pp.tile([128, 16], DT.float32, tag="sums")
            sqs = pp.tile([128, 16], DT.float32, tag="sqs")
            junk = pp.tile([128, R8 * W], DT.float16, tag="junk")
            wtv = wt_sb[:].rearrange("c (t o) -> c t o", o=O)

            for g2 in range(NG2):
                pacc = [ppacc.tile([128, R8 * W], DT.float32, tag="acc", name=f"pacc{i}")
                        for i in range(2)]
                for n in range(NT):
                    call2 = g2 * NT + n
                    gt = []
                    for r in range(2):
                        g_t = pg.tile([128, R8, 2 * C], DT.float16,
                                      tag=f"g{r}", name=f"g_t{r}")
                        gi = nc.gpsimd.dma_gather(
                            out_ap=g_t[:], in_ap=src_ap,
                            idxs_ap=wrap[:, call2 * 128 + r * 64:
                                         call2 * 128 + (r + 1) * 64],
                            num_idxs=R8 * W, num_idxs_reg=R8 * W,
                            elem_size=2 * C, elem_step=C)
                        for st in xpt_stores:
                            add_dep_helper(gi.ins, st.ins, sync=True,
                                           reason="gather after xpt store")
                        gt.append(g_t[:])
                    st0 = [pst.tile([128, R8 * W // 2], DT.float16,
                                    tag=f"st0{i}", name=f"st0{i}")
                           for i in range(2)]
                    pt = pps.tile([128, R8 * W], DT.float16, tag="tph")
                    for jj in range(R8):
                        col = (g2 * R8 + jj) * NT + n
                        s0 = st0[jj % 2]
                        sl = slice((jj // 2) * 128, (jj // 2 + 1) * 128)
                        psl = slice(jj * 128, (jj + 1) * 128)
                        nc.scalar.activation(out=s0[:, sl],
                                             in_=gt[0][:, jj, 0:C],
                                             func=AF.Copy,
                                             scale=wlt[:, col:col + 1])
                        for wgt, g_t, cs in ((wlb, gt[0], slice(C, 2 * C)),
                                             (wrt, gt[1], slice(0, C)),
                                             (wrb, gt[1], slice(C, 2 * C))):
                            nc.vector.scalar_tensor_tensor(
                                out=s0[:, sl], in0=g_t[:, jj, cs],
                                scalar=wgt[:, col:col + 1], in1=s0[:, sl],
                                op0=ALU.mult, op1=ALU.add)
                        nc.tensor.transpose(out=pt[:, psl], in_=s0[:, sl],
                                            identity=idh[:])
                    rhs16 = pst.tile([128, R8 * W], DT.float16, tag="rhs16")
                    nc.scalar.copy(rhs16[:], pt[:])
                    for oc in range(2):
                        for hh in range(2):
                            sl = slice(hh * 512, (hh + 1) * 512)
                            nc.tensor.matmul(pacc[oc][:, sl],
                                             lhsT=wtv[:, n, oc * 128:(oc + 1) * 128],
                                             rhs=rhs16[:, sl],
                                             start=(n == 0), stop=(n == 8))
                for oc in range(2):
                    seg = slice(oc * NPOS + g2 * R8 * W, oc * NPOS + (g2 + 1) * R8 * W)
                    nc.scalar.activation(out=out_sb[:, seg], in_=pacc[oc][:],
                                         func=AF.Copy,
                                         accum_out=sums[:, oc * 8 + g2:oc * 8 + g2 + 1])
                    nc.scalar.activation(out=junk[:], in_=pacc[oc][:],
                                         func=AF.Square,
                                         accum_out=sqs[:, oc * 8 + g2:oc * 8 + g2 + 1])

            # ---------------- P5: BN stats + collective ----------------------
            stats = pp.tile([128, 4], DT.float32, tag="stats")
            nc.vector.tensor_reduce(out=stats[:, 0:1], in_=sums[:, 0:8],
                                    axis=mybir.AxisListType.X, op=ALU.add)
            nc.vector.tensor_reduce(out=stats[:, 1:2], in_=sqs[:, 0:8],
                                    axis=mybir.AxisListType.X, op=ALU.add)
            nc.vector.tensor_reduce(out=stats[:, 2:3], in_=sums[:, 8:16],
                                    axis=mybir.AxisListType.X, op=ALU.add)
            nc.vector.tensor_reduce(out=stats[:, 3:4], in_=sqs[:, 8:16],
                                    axis=mybir.AxisListType.X, op=ALU.add)
            d1 = nc.sync.dma_start(out=cc_in[:], in_=stats[:])
            if with_collective:
                cci = nc.gpsimd.collective_compute(
                    "AllReduce", ALU.add,
                    replica_groups=[list(range(N_CORES))],
                    ins=[cc_in[:].opt()], outs=[cc_out[:].opt()])
            else:
                cci = nc.sync.dma_start(out=cc_out[:], in_=cc_in[:])
            add_dep_helper(cci.ins, d1.ins, sync=True, reason="cc after stats store")
            ast = pp.tile([128, 4], DT.float32, tag="ast")
            d2 = nc.sync.dma_start(out=ast[:], in_=cc_out[:])
            add_dep_helper(d2.ins, cci.ins, sync=True, reason="readback after cc")

            astv = ast[:].rearrange("p (a b) -> p a b", b=2)
            cnt = float(B * H * W)
            mean = pp.tile([128, 2], DT.float32, tag="mean")
            nc.vector.tensor_scalar(out=mean[:], in0=astv[:, :, 0], scalar1=1.0 / cnt,
                                    scalar2=None, op0=ALU.mult)
            var = pp.tile([128, 2], DT.float32, tag="var")
            nc.vector.tensor_scalar(out=var[:], in0=astv[:, :, 1], scalar1=1.0 / cnt,
                                    scalar2=None, op0=ALU.mult)
            msq = pp.tile([128, 2], DT.float32, tag="msq")
            nc.vector.tensor_tensor(out=msq[:], in0=mean[:], in1=mean[:], op=ALU.mult)
            nc.vector.tensor_tensor(out=var[:], in0=var[:], in1=msq[:],
                                    op=ALU.subtract)
            epsb = pp.tile([128, 1], DT.float32, tag="epsb")
            nc.vector.memset(epsb[:], EPS)
            std = pp.tile([128, 2], DT.float32, tag="std")
            nc.scalar.activation(out=std[:], in_=var[:], func=AF.Sqrt, bias=epsb[:])
            rstd = pp.tile([128, 2], DT.float32, tag="rstd")
            nc.vector.reciprocal(rstd[:], std[:])
            sc = pp.tile([128, 2], DT.float32, tag="sc")
            nc.vector.tensor_tensor(out=sc[:], in0=rstd[:], in1=gam_sb[:],
                                    op=ALU.mult)
            bb = pp.tile([128, 2], DT.float32, tag="bb")
            nc.vector.tensor_tensor(out=bb[:], in0=mean[:], in1=sc[:], op=ALU.mult)
            nc.vector.tensor_tensor(out=bb[:], in0=bet_sb[:], in1=bb[:],
                                    op=ALU.subtract)
            sc9 = pp.tile([128, 2], DT.float32, tag="sc9")
            bb9 = pp.tile([128, 2], DT.float32, tag="bb9")
            sc1 = pp.tile([128, 2], DT.float32, tag="sc1")
            bb1 = pp.tile([128, 2], DT.float32, tag="bb1")
            for dst_t, src_t, f in ((sc9, sc, 1 - LEAK), (bb9, bb, 1 - LEAK),
                                    (sc1, sc, LEAK), (bb1, bb, LEAK)):
                nc.vector.tensor_scalar(out=dst_t[:], in0=src_t[:], scalar1=float(f),
                                        scalar2=None, op0=ALU.mult)

            # ---------------- P6: normalize + LeakyReLU + store --------------
            SEG = 1024
            for oc in range(2):
                for s in range(NPOS // SEG):
                    seg = slice(oc * NPOS + s * SEG, oc * NPOS + (s + 1) * SEG)
                    y1 = pst.tile([128, SEG], DT.float32, tag="y1")
                    rr = pst.tile([128, SEG], DT.float32, tag="rr")
                    nc.scalar.activation(out=y1[:], in_=out_sb[:, seg],
                                         func=AF.Identity,
                                         scale=sc1[:, oc:oc + 1],
                                         bias=bb1[:, oc:oc + 1])
                    nc.scalar.activation(out=rr[:], in_=out_sb[:, seg],
                                         func=AF.Relu,
                                         scale=sc9[:, oc:oc + 1],
                                         bias=bb9[:, oc:oc + 1])
                    nc.vector.tensor_tensor(out=y1[:], in0=y1[:], in1=rr[:],
                                            op=ALU.add)
                    nc.sync.dma_start(out=out_d[oc, :, s * SEG:(s + 1) * SEG],
                                      in_=y1[:])

    nc.compile()
    return nc


# ---------------------------------------------------------------------------
# host side
# ---------------------------------------------------------------------------
def prep_in_maps(x, p_w, p_b, w_conv, gamma, beta):
    x = np.asarray(x, np.float32)
    p_w = np.asarray(p_w, np.float32)
    p_b = np.asarray(p_b, np.float32)
    w_conv = np.asarray(w_conv, np.float32)
    gamma = np.asarray(gamma, np.float32)
    beta = np.asarray(beta, np.float32)

    pwT = np.stack([p_w[:, :, t // 3, t % 3].T for t in range(NT)]) \
        .astype(np.float16)                                      # (9, C, 18)
    wT = np.stack([w_conv[:, :, t // 3, t % 3].T for t in range(NT)]) \
        .astype(np.float16)                                      # (9, C, O)
    pb = p_b.reshape(2 * NT, 1).astype(np.float32)
    gamma2 = np.ascontiguousarray(gamma.reshape(2, 128).T)
    beta2 = np.ascontiguousarray(beta.reshape(2, 128).T)
    identf = np.eye(128, dtype=np.float32)
    identh = np.eye(128, dtype=np.float16)

    rr = np.arange(ROWS, dtype=np.float32)[:, None]
    ww = np.arange(W, dtype=np.float32)[:, None, None]
    by = np.broadcast_to((1 + ww + DY[None, None, :]),
                         (W, ROWS, NT)).reshape(W, CH).astype(np.float32)

    in_maps = []
    for core in range(N_CORES):
        bi, half = core // 2, core % 2
        h0 = 64 * half
        xw = np.zeros((C, 66, W), np.float32)
        lo, hi = h0 - 1, h0 + 65
        glo, ghi = max(lo, 0), min(hi, H)
        xw[:, glo - lo:glo - lo + (ghi - glo)] = x[bi, :, glo:ghi]
        ax = np.broadcast_to((h0 + 1 + rr + DX[None, :]),
                             (ROWS, NT)).reshape(1, CH)
        ax = np.broadcast_to(ax, (128, CH)).astype(np.float32)
        in_maps.append({
            "x_img": np.ascontiguousarray(x[bi].reshape(C, H * W)),
            "x_win": np.ascontiguousarray(xw.reshape(C, 66 * W)),
            "pwT": pwT, "pb": pb, "wT": wT,
            "Ax": np.ascontiguousarray(ax), "By": np.ascontiguousarray(by),
            "gamma2": gamma2, "beta2": beta2,
            "identf": identf, "identh": identh,
        })
    return in_maps


def assemble(results):
    out = np.zeros((B, O, H, W), np.float32)
    for core, om in enumerate(results):
        bi, half = core // 2, core % 2
        h0 = 64 * half
        oc = np.asarray(om["out"]).reshape(O, ROWS, W)
        out[bi, :, h0:h0 + 64, :] = oc
    return out


_NC_CACHE = {}


def _get_nc(with_collective=True):
    key = with_collective
    if key not in _NC_CACHE:
        _NC_CACHE[key] = build_kernel(with_collective)
    return _NC_CACHE[key]


def kernel(**inputs):
    from concourse.bass_utils import run_bass_kernel_spmd
    nc = _get_nc(True)
    in_maps = prep_in_maps(**inputs)
    res = run_bass_kernel_spmd(nc, in_maps, core_ids=list(range(N_CORES)))
    return assemble(res.results)


if __name__ == "__main__":
    build_kernel(False)
    print("build ok")
